# revision 3
# baseline (speedup 1.0000x reference)
"""AdaIN statistics kernel for TRN2, SPMD across 8 NeuronCores.

Input : f_vol [32, 512, 64, 64] f32
Output: [32, 1024] f32 = concat([mean over (h,w), unbiased std over (h,w)], axis=-1)

Data-parallel over batch: each core handles 4 batches = 2048 rows x 4096 f32.

Structure (per core):
- Mid-stream: 6 interleaved m=2 slabs (U0-U5, groups g0..g11). Partition p
  holds rows base+2p, base+2p+1 (one 32-KiB descriptor per partition).
  U1, U3 are consumed by the Scalar (ACT) engine via Copy/Square+accumulate;
  the rest by DVE bn_stats. Each mid slab's (mean,std) go straight to DRAM
  with a small interleaved 3D-AP DMA whose completion latency hides
  mid-stream.
- Endgame: the last 4 row groups are fetched group-aligned (partition p =
  row g*128+p) and column-split so that BOTH engines chase the stream to
  its very end:
    g12: one DMA, 8 bn_stats on DVE.
    g13/g14: DMA B (cols 2560:4096) consumed by ACT (sum/sumsq via
      accumulate), then DMA A (cols 0:2560) consumed by DVE (5 bn_stats).
      ACT converts (sum,sumsq) into a synthetic bn_stats entry (two
      (count,mean,M2) triples of 768 elements each) so that a single
      bn_aggr over 6 entries produces the full-row mean/var exactly.
    g15: B plus A split again into A1 (0:1536) and A2 (1536:2560) so the
      final bytes to arrive need only 2 bn_stats before the aggregate.
- Output endgame: scalar epilogues write mean/std columns into F[128,32]
  (col e = mean g12+e, col 16+e = std); DVE block-transposes F -> T; two
  512-float contiguous DMAs write out[3, 0:512] and out[3, 512:1024].

Raw Bass with manual semaphores; every cross-engine data edge is covered
by an explicit semaphore observation.
"""

from contextlib import ExitStack

import numpy as np

B, C, H, W = 32, 512, 64, 64
N_CORES = 8
B_LOCAL = B // N_CORES  # 4
N = H * W  # 4096
P = 128
ROWS = B_LOCAL * C  # 2048
G = N // 512  # bn_stats groups per full row = 8
NBUF = 6  # xt ring slots of 2*N floats

A_COLS = 2560  # DVE part of split groups (5 bn groups)
B_COLS = N - A_COLS  # 1536, ACT part
A1_COLS = 1536  # g15 sub-chunk 1 (3 bn groups)
A2_COLS = A_COLS - A1_COLS  # 1024 (2 bn groups)

_CACHE = {}


def _build():
    import concourse.bass as bass
    from concourse import mybir

    f32 = mybir.dt.float32
    AF = mybir.ActivationFunctionType

    nc = bass.Bass()
    x_ext = nc.declare_dram_parameter(
        "f_vol", [B_LOCAL, C, H, W], f32, isOutput=False
    )
    out_ext = nc.declare_dram_parameter("out", [B_LOCAL, 2 * C], f32, isOutput=True)

    x = x_ext.ap().rearrange("b c h w -> (b c) (h w)")  # [2048, 4096]

    # mid slabs U0..U5: rows 256*u .. 256*u+256, consumer DVE except U1,U3
    MID_ACT = {1, 3}
    # endgame units U6..U9 = groups 12..15
    EG_BASE = 1536  # first endgame row

    NA = 1.0 / np.sqrt(float(N) * (N - 1))  # for mid ACT epilogue

    with ExitStack() as ctx:
        block = ctx.enter_context(nc.Block(no_gpsimd_drain=True))
        # input completion sems: 6 mid + g12 + g13{B,A} + g14{B,A} + g15{B,A1,A2}
        dmid = [ctx.enter_context(nc.semaphore(f"dmid{u}")) for u in range(6)]
        d12B = ctx.enter_context(nc.semaphore("d12B"))
        d12A = ctx.enter_context(nc.semaphore("d12A"))
        d13B = ctx.enter_context(nc.semaphore("d13B"))
        d13A = ctx.enter_context(nc.semaphore("d13A"))
        d14B = ctx.enter_context(nc.semaphore("d14B"))
        d14A = ctx.enter_context(nc.semaphore("d14A"))
        d15B = ctx.enter_context(nc.semaphore("d15B"))
        d15A1 = ctx.enter_context(nc.semaphore("d15A1"))
        d15A2 = ctx.enter_context(nc.semaphore("d15A2"))
        d15A3 = ctx.enter_context(nc.semaphore("d15A3"))
        dve_stats = ctx.enter_context(nc.semaphore("dve_stats"))
        mv_ready = ctx.enter_context(nc.semaphore("mv_ready"))
        act_stats = ctx.enter_context(nc.semaphore("act_stats"))
        act_done = ctx.enter_context(nc.semaphore("act_done"))
        trans_done = ctx.enter_context(nc.semaphore("trans_done"))
        dma_out = ctx.enter_context(nc.semaphore("dma_out"))
        dma_fin = ctx.enter_context(nc.semaphore("dma_fin"))

        xt = ctx.enter_context(nc.sbuf_tensor("xt", [P, NBUF, 2 * N], f32))
        # mid DVE slabs (U0,U2,U4,U5 -> idx 0..3): stats + (mean,var)
        stats = ctx.enter_context(nc.sbuf_tensor("stats", [P, 4, 2, G, 6], f32))
        mv = ctx.enter_context(nc.sbuf_tensor("mv", [P, 4, 2, 2], f32))
        # per-mid-slab output staging [mean|std rows x m]
        res = ctx.enter_context(nc.sbuf_tensor("res", [P, 6, 2, 2], f32))
        # mid ACT slabs (U1,U3): [slab, row, {sum,sumsq,tmp}]
        accM = ctx.enter_context(nc.sbuf_tensor("accM", [P, 2, 2, 3], f32))
        # endgame: statsE[e, entry, 6] for e = g12..g15; entry 5 = synthetic
        statsE = ctx.enter_context(nc.sbuf_tensor("statsE", [P, 4, 8, 6], f32))
        mvE = ctx.enter_context(nc.sbuf_tensor("mvE", [P, 4, 2], f32))
        accE = ctx.enter_context(nc.sbuf_tensor("accE", [P, 4, 3], f32))
        scr = ctx.enter_context(nc.sbuf_tensor("scr", [P, B_COLS], f32))
        F = ctx.enter_context(nc.sbuf_tensor("F", [P, 32], f32))
        T = ctx.enter_context(nc.sbuf_tensor("T", [P, P], f32))

        # ---- cumulative semaphore targets ----
        # dve_stats: mid DVE slabs 16 each, then 8 (g12), 5 (g13), 5 (g14), 5 (g15)
        DS_MID = {0: 16, 2: 32, 4: 48, 5: 64}
        DS_G12, DS_G13, DS_G14, DS_G15 = 69, 74, 79, 84
        # mv_ready: 2 per mid DVE slab, 1 per endgame group
        MV_MID = {0: 2, 2: 4, 4: 6, 5: 8}
        MV_G12, MV_G13, MV_G14, MV_G15 = 9, 10, 11, 12
        # act_stats: mid ACT slabs 4 each (2 rows x Copy/Square);
        # endgame B passes 8 each (Copy,Sq,m2a,m2b,t,neg,M2a,M2b)
        AS_U1, AS_U3 = 4, 8
        AS_G12, AS_G13, AS_G14, AS_G15 = 15, 22, 29, 36
        # act_done (scalar program order):
        # U0 epi 4; U1 epi 8 -> 12; U2 epi 16; U3 epi 24; U4 28; U5 32;
        # endgame epis: 2 strided ops -> 34
        AD_TOTAL = 34

        @block.sync
        def _(sync):
            def mid_src(u):
                return x[u * 2 * P : (u + 1) * 2 * P, :].rearrange(
                    "(p m) f -> p (m f)", m=2
                )

            def eg_src(g, c0, w):
                return bass.AP(
                    tensor=x_ext,
                    offset=(EG_BASE + (g - 12) * P) * N + c0,
                    ap=[[N, P], [1, w]],
                )

            for u in range(6):
                sync.dma_start(out=xt[:, u, 0 : 2 * N], in_=mid_src(u)).then_inc(
                    dmid[u], 16
                )
            # endgame units reuse ring slots 0..3 of U0..U3
            # g12 -> slot 0 (U0, DVE-consumed)
            sync.wait_ge(dve_stats, DS_MID[0])
            sync.wait_ge(dmid[0], 16)
            sync.dma_start(
                out=xt[:, 0, A_COLS:N], in_=eg_src(12, A_COLS, B_COLS)
            ).then_inc(d12B, 16)
            sync.dma_start(out=xt[:, 0, 0:A_COLS], in_=eg_src(12, 0, A_COLS)).then_inc(
                d12A, 16
            )
            # g13 -> slot 1 (U1, ACT-consumed): B then A
            sync.wait_ge(act_stats, AS_U1)
            sync.wait_ge(dmid[1], 16)
            sync.dma_start(
                out=xt[:, 1, A_COLS:N], in_=eg_src(13, A_COLS, B_COLS)
            ).then_inc(d13B, 16)
            sync.dma_start(out=xt[:, 1, 0:A_COLS], in_=eg_src(13, 0, A_COLS)).then_inc(
                d13A, 16
            )
            # g14 -> slot 2 (U2, DVE)
            sync.wait_ge(dve_stats, DS_MID[2])
            sync.wait_ge(dmid[2], 16)
            sync.dma_start(
                out=xt[:, 2, A_COLS:N], in_=eg_src(14, A_COLS, B_COLS)
            ).then_inc(d14B, 16)
            sync.dma_start(out=xt[:, 2, 0:A_COLS], in_=eg_src(14, 0, A_COLS)).then_inc(
                d14A, 16
            )
            # g15 -> slot 3 (U3, ACT): B, A1, A2
            sync.wait_ge(act_stats, AS_U3)
            sync.wait_ge(dmid[3], 16)
            sync.dma_start(
                out=xt[:, 3, A_COLS:N], in_=eg_src(15, A_COLS, B_COLS)
            ).then_inc(d15B, 16)
            sync.dma_start(
                out=xt[:, 3, 0:1024], in_=eg_src(15, 0, 1024)
            ).then_inc(d15A1, 16)
            sync.dma_start(
                out=xt[:, 3, 1024:2048], in_=eg_src(15, 1024, 1024)
            ).then_inc(d15A2, 16)
            sync.dma_start(
                out=xt[:, 3, 2048:A_COLS], in_=eg_src(15, 2048, 512)
            ).then_inc(d15A3, 16)

            # final output DMAs for g12..g15 via T
            sync.wait_ge(trans_done, 5)
            dst = bass.AP(tensor=out_ext, offset=3 * 2 * C, ap=[[C, 2], [1, C]])
            sync.dma_start(out=dst, in_=T[0:8, 0:P]).then_inc(dma_fin, 16)
            sync.wait_ge(dma_out, 6 * 16)
            sync.wait_ge(dma_fin, 16)

        @block.vector
        def _(vector):
            # init: zero statsE (incl. synthetic zero fields), synth counts=768,
            # zero F. trans_done=1 signals all init memsets retired.
            vector.memset(statsE[:, :, :, :], 0.0)
            for e in range(4):
                vector.memset(statsE[:, e, 5, 0:1], float(B_COLS // 2))
                vector.memset(statsE[:, e, 5, 3:4], float(B_COLS // 2))
            vector.memset(F[:, :], 0.0).then_inc(trans_done, 1)

            ds = 0
            # mid DVE slabs
            for si, u in enumerate([0, 2, 4, 5]):
                vector.wait_ge(dmid[u], 16)
                for r in range(2):
                    for g in range(G):
                        vector.bn_stats(
                            out=stats[:, si, r, g, :],
                            in_=xt[:, u, (r * G + g) * 512 : (r * G + g + 1) * 512],
                        ).then_inc(dve_stats, 1)
                ds += 16
                vector.wait_ge(dve_stats, ds)
                for r in range(2):
                    vector.bn_aggr(
                        out=mv[:, si, r, :], in_=stats[:, si, r, :, :]
                    ).then_inc(mv_ready, 1)

            # g12 / g13 / g14 A-parts
            for e, (dA, slot, as_tgt) in enumerate(
                [(d12A, 0, AS_G12), (d13A, 1, AS_G13), (d14A, 2, AS_G14)]
            ):
                vector.wait_ge(dA, 16)
                for g in range(5):
                    vector.bn_stats(
                        out=statsE[:, e, g, :],
                        in_=xt[:, slot, g * 512 : (g + 1) * 512],
                    ).then_inc(dve_stats, 1)
                ds += 5
                vector.wait_ge(dve_stats, ds)
                vector.wait_ge(act_stats, as_tgt)  # synthetic entry written
                vector.bn_aggr(out=mvE[:, e, :], in_=statsE[:, e, 0:6, :]).then_inc(
                    mv_ready, 1
                )

            # g15: A chunks [1024, 1024, 512]
            for dchunk, ks in [(d15A1, (0, 2)), (d15A2, (2, 4)), (d15A3, (4, 5))]:
                vector.wait_ge(dchunk, 16)
                for g in range(*ks):
                    vector.bn_stats(
                        out=statsE[:, 3, g, :], in_=xt[:, 3, g * 512 : (g + 1) * 512]
                    ).then_inc(dve_stats, 1)
            ds += 5
            vector.wait_ge(dve_stats, ds)
            vector.wait_ge(act_stats, AS_G15)
            vector.bn_aggr(out=mvE[:, 3, :], in_=statsE[:, 3, 0:6, :]).then_inc(
                mv_ready, 1
            )

            # transposes F -> T once every endgame epilogue has written F
            vector.wait_ge(act_done, AD_TOTAL)
            for blk in range(P // 32):
                vector.transpose(
                    out=T[0:32, blk * 32 : blk * 32 + 32],
                    in_=F[blk * 32 : blk * 32 + 32, 0:32],
                ).then_inc(trans_done, 1)

        @block.scalar
        def _(scalar):
            ad = [0]  # running act_done
            ast = [0]  # running act_stats

            def mid_dve_epi(u, si, mv_tgt):
                scalar.wait_ge(mv_ready, mv_tgt)
                for r in range(2):
                    scalar.copy(out=res[:, u, 0, r : r + 1], in_=mv[:, si, r, 0:1]).then_inc(
                        act_done, 1
                    )
                    scalar.activation(
                        out=res[:, u, 1, r : r + 1],
                        in_=mv[:, si, r, 1:2],
                        func=AF.Sqrt,
                        scale=float(N) / (N - 1),
                    ).then_inc(act_done, 1)
                ad[0] += 4

            def mid_out_dma(u):
                b, c0 = divmod(u * 2 * P, C)
                scalar.wait_ge(act_done, ad[0])
                dst = bass.AP(
                    tensor=out_ext,
                    offset=b * 2 * C + c0,
                    ap=[[2, P], [C, 2], [1, 2]],
                )
                scalar.dma_start(out=dst, in_=res[:, u, :, 0:2]).then_inc(dma_out, 16)

            def mid_act(u, k):
                scalar.wait_ge(dmid[u], 16)
                for r in range(2):
                    row = xt[:, u, r * N : (r + 1) * N]
                    scalar.activation(
                        out=row, in_=row, func=AF.Copy, accum_out=accM[:, k, r, 0:1]
                    ).then_inc(act_stats, 1)
                    ast[0] += 1
                    scalar.wait_ge(act_stats, ast[0])
                    scalar.activation(
                        out=row, in_=row, func=AF.Square, accum_out=accM[:, k, r, 1:2]
                    ).then_inc(act_stats, 1)
                    ast[0] += 1
                # epilogue from raw sums
                for r in range(2):
                    scalar.activation(
                        out=res[:, u, 0, r : r + 1],
                        in_=accM[:, k, r, 0:1],
                        func=AF.Copy,
                        scale=1.0 / N,
                    ).then_inc(act_done, 1)
                    scalar.activation(
                        out=accM[:, k, r, 2:3],
                        in_=accM[:, k, r, 0:1],
                        func=AF.Square,
                        scale=NA,
                    ).then_inc(act_done, 1)
                    ad[0] += 2
                    scalar.wait_ge(act_done, ad[0])
                    scalar.activation(
                        out=accM[:, k, r, 2:3],
                        in_=accM[:, k, r, 2:3],
                        func=AF.Copy,
                        scale=-1.0,
                    ).then_inc(act_done, 1)
                    ad[0] += 1
                    scalar.wait_ge(act_done, ad[0])
                    scalar.activation(
                        out=res[:, u, 1, r : r + 1],
                        in_=accM[:, k, r, 1:2],
                        func=AF.Sqrt,
                        scale=1.0 / (N - 1),
                        bias=accM[:, k, r, 2:3],
                    ).then_inc(act_done, 1)
                    ad[0] += 1

            def eg_b_pass(e, dB, slot):
                """B-part accumulate + synthetic bn_stats entry for group 12+e."""
                k = e  # accE index
                scalar.wait_ge(dB, 16)
                seg = xt[:, slot, A_COLS:N]
                scalar.activation(
                    out=scr[:, :], in_=seg, func=AF.Copy, accum_out=accE[:, k, 0:1]
                ).then_inc(act_stats, 1)
                # Square with scale 1/sqrt(2) accumulates sumsqB/2 directly
                scalar.activation(
                    out=scr[:, :],
                    in_=seg,
                    func=AF.Square,
                    scale=1.0 / np.sqrt(2.0),
                    accum_out=accE[:, k, 1:2],
                ).then_inc(act_stats, 1)
                ast[0] += 2
                # synthetic entry: two triples [768, meanB, M2B/2]
                if e == 0:
                    scalar.wait_ge(trans_done, 1)  # statsE memsets retired
                scalar.wait_ge(act_stats, ast[0])  # sums retired
                scalar.activation(
                    out=statsE[:, e, 5, 1:2],
                    in_=accE[:, k, 0:1],
                    func=AF.Copy,
                    scale=1.0 / B_COLS,
                ).then_inc(act_stats, 1)
                scalar.activation(
                    out=statsE[:, e, 5, 4:5],
                    in_=accE[:, k, 0:1],
                    func=AF.Copy,
                    scale=1.0 / B_COLS,
                ).then_inc(act_stats, 1)
                # tmp = sumB^2 / (2*B_COLS)
                scalar.activation(
                    out=accE[:, k, 2:3],
                    in_=accE[:, k, 0:1],
                    func=AF.Square,
                    scale=1.0 / np.sqrt(2.0 * B_COLS),
                ).then_inc(act_stats, 1)
                ast[0] += 3
                scalar.wait_ge(act_stats, ast[0])
                # M2B/2 = sumsqB/2 - sumB^2/(2*B_COLS)
                scalar.activation(
                    out=statsE[:, e, 5, 2:3],
                    in_=accE[:, k, 2:3],
                    func=AF.Identity,
                    scale=-1.0,
                    bias=accE[:, k, 1:2],
                ).then_inc(act_stats, 1)
                scalar.activation(
                    out=statsE[:, e, 5, 5:6],
                    in_=accE[:, k, 2:3],
                    func=AF.Identity,
                    scale=-1.0,
                    bias=accE[:, k, 1:2],
                ).then_inc(act_stats, 1)
                ast[0] += 2

            def eg_epis():
                # all four endgame groups in two strided ops
                scalar.wait_ge(mv_ready, MV_G15)
                scalar.copy(
                    out=F[:, 0:4],
                    in_=mvE[:, 0:4, 0:1].rearrange("p a b -> p (a b)"),
                ).then_inc(act_done, 1)
                scalar.activation(
                    out=F[:, 4:8],
                    in_=mvE[:, 0:4, 1:2].rearrange("p a b -> p (a b)"),
                    func=AF.Sqrt,
                    scale=float(N) / (N - 1),
                ).then_inc(act_done, 1)
                ad[0] += 2

            mid_dve_epi(0, 0, MV_MID[0])
            mid_out_dma(0)
            mid_act(1, 0)
            mid_out_dma(1)
            mid_dve_epi(2, 1, MV_MID[2])
            mid_out_dma(2)
            mid_act(3, 1)
            mid_out_dma(3)
            mid_dve_epi(4, 2, MV_MID[4])
            mid_out_dma(4)
            # endgame: B passes take priority over epilogues
            eg_b_pass(0, d12B, 0)
            mid_dve_epi(5, 3, MV_MID[5])
            mid_out_dma(5)
            eg_b_pass(1, d13B, 1)
            eg_b_pass(2, d14B, 2)
            eg_b_pass(3, d15B, 3)
            eg_epis()
            assert ad[0] == AD_TOTAL, ad[0]
            assert ast[0] == AS_G15, ast[0]

    return nc


def kernel(f_vol: np.ndarray) -> np.ndarray:
    from concourse.bass_utils import run_bass_kernel_spmd

    if "nc" not in _CACHE:
        _CACHE["nc"] = _build()
    nc = _CACHE["nc"]

    f_vol = np.ascontiguousarray(f_vol, dtype=np.float32)
    in_maps = [
        {"f_vol": f_vol[i * B_LOCAL : (i + 1) * B_LOCAL]} for i in range(N_CORES)
    ]
    res = run_bass_kernel_spmd(nc, in_maps, core_ids=list(range(N_CORES)))
    return np.concatenate([res.results[i]["out"] for i in range(N_CORES)], axis=0)


# revision 4
# speedup vs baseline: 1.4568x; 1.4568x over previous
"""AdaIN statistics kernel for TRN2, SPMD across 8 NeuronCores.

Input : f_vol [32, 512, 64, 64] f32
Output: [32, 1024] f32 = concat([mean over (h,w), unbiased std over (h,w)], axis=-1)

Data-parallel over batch: each core handles 4 batches = 2048 rows x 4096 f32.

Structure (per core):
- Mid-stream: 6 interleaved m=2 slabs (U0-U5, groups g0..g11). Partition p
  holds rows base+2p, base+2p+1 (one 32-KiB descriptor per partition).
  U1, U3 are consumed by the Scalar (ACT) engine via Copy/Square+accumulate;
  the rest by DVE bn_stats. Each mid slab's (mean,std) go straight to DRAM
  with a small interleaved 3D-AP DMA whose completion latency hides
  mid-stream.
- Endgame: the last 4 row groups (g12..g15) are fetched group-aligned
  (partition p = row g*128+p) and column-split so that BOTH engines chase
  the stream to its very end: DMA B (cols 2560:4096) is consumed by ACT
  (sum and sumsq/2 via Copy/Square+accumulate), then DMA A (cols 0:2560)
  by DVE (5x bn_stats). ACT converts (sum, sumsq/2) into a synthetic
  bn_stats entry (two (count=768, meanB, M2B/2) triples), so one bn_aggr
  over 6 entries produces the exact full-row mean/var. g15's A part is
  further split into chunks [1024, 1024, 512] so the final bytes to
  arrive need only one bn_stats before the aggregate.
- Output endgame: two strided scalar ops write the four means into
  F[:, 0:4] and stds into F[:, 4:8]; DVE block-transposes F -> T; ONE
  DMA (T[0:8] -> out[3, :], 2-KiB runs) writes the last batch row.

Raw Bass with manual semaphores; every cross-engine data edge is covered
by an explicit semaphore observation.
"""

from contextlib import ExitStack

import numpy as np

B, C, H, W = 32, 512, 64, 64
N_CORES = 8
B_LOCAL = B // N_CORES  # 4
N = H * W  # 4096
P = 128
ROWS = B_LOCAL * C  # 2048
G = N // 512  # bn_stats groups per full row = 8
NBUF = 6  # xt ring slots of 2*N floats

A_COLS = 2560  # DVE part of split groups (5 bn groups)
B_COLS = N - A_COLS  # 1536, ACT part
A1_COLS = 1536  # g15 sub-chunk 1 (3 bn groups)
A2_COLS = A_COLS - A1_COLS  # 1024 (2 bn groups)

_CACHE = {}


def _build():
    import concourse.bass as bass
    from concourse import mybir

    f32 = mybir.dt.float32
    AF = mybir.ActivationFunctionType

    nc = bass.Bass()
    x_ext = nc.declare_dram_parameter(
        "f_vol", [B_LOCAL, C, H, W], f32, isOutput=False
    )
    out_ext = nc.declare_dram_parameter("out", [B_LOCAL, 2 * C], f32, isOutput=True)

    x = x_ext.ap().rearrange("b c h w -> (b c) (h w)")  # [2048, 4096]

    # mid slabs U0..U5: rows 256*u .. 256*u+256, consumer DVE except U1,U3
    MID_ACT = {1, 3}
    # endgame units U6..U9 = groups 12..15
    EG_BASE = 1536  # first endgame row

    NA = 1.0 / np.sqrt(float(N) * (N - 1))  # for mid ACT epilogue

    with ExitStack() as ctx:
        block = ctx.enter_context(nc.Block(no_gpsimd_drain=True))
        # input completion sems: 6 mid + g12 + g13{B,A} + g14{B,A} + g15{B,A1,A2}
        dmid = [ctx.enter_context(nc.semaphore(f"dmid{u}")) for u in range(6)]
        d12B = ctx.enter_context(nc.semaphore("d12B"))
        d12A = ctx.enter_context(nc.semaphore("d12A"))
        d13B = ctx.enter_context(nc.semaphore("d13B"))
        d13A = ctx.enter_context(nc.semaphore("d13A"))
        d14B = ctx.enter_context(nc.semaphore("d14B"))
        d14A = ctx.enter_context(nc.semaphore("d14A"))
        d15B = ctx.enter_context(nc.semaphore("d15B"))
        d15A1 = ctx.enter_context(nc.semaphore("d15A1"))
        d15A2 = ctx.enter_context(nc.semaphore("d15A2"))
        d15A3 = ctx.enter_context(nc.semaphore("d15A3"))
        dve_stats = ctx.enter_context(nc.semaphore("dve_stats"))
        mv_ready = ctx.enter_context(nc.semaphore("mv_ready"))
        act_stats = ctx.enter_context(nc.semaphore("act_stats"))
        act_done = ctx.enter_context(nc.semaphore("act_done"))
        trans_done = ctx.enter_context(nc.semaphore("trans_done"))
        dma_out = ctx.enter_context(nc.semaphore("dma_out"))
        dma_fin = ctx.enter_context(nc.semaphore("dma_fin"))

        xt = ctx.enter_context(nc.sbuf_tensor("xt", [P, NBUF, 2 * N], f32))
        # mid DVE slabs (U0,U2,U4,U5 -> idx 0..3): stats + (mean,var)
        stats = ctx.enter_context(nc.sbuf_tensor("stats", [P, 4, 2, G, 6], f32))
        mv = ctx.enter_context(nc.sbuf_tensor("mv", [P, 4, 2, 2], f32))
        # per-mid-slab output staging [mean|std rows x m]
        res = ctx.enter_context(nc.sbuf_tensor("res", [P, 6, 2, 2], f32))
        # mid ACT slabs (U1,U3): [slab, row, {sum,sumsq,tmp}]
        accM = ctx.enter_context(nc.sbuf_tensor("accM", [P, 2, 2, 3], f32))
        # endgame: statsE[e, entry, 6] for e = g12..g15; entry 5 = synthetic
        statsE = ctx.enter_context(nc.sbuf_tensor("statsE", [P, 4, 8, 6], f32))
        mvE = ctx.enter_context(nc.sbuf_tensor("mvE", [P, 4, 2], f32))
        accE = ctx.enter_context(nc.sbuf_tensor("accE", [P, 4, 3], f32))
        scr = ctx.enter_context(nc.sbuf_tensor("scr", [P, B_COLS], f32))
        F = ctx.enter_context(nc.sbuf_tensor("F", [P, 32], f32))
        T = ctx.enter_context(nc.sbuf_tensor("T", [P, P], f32))

        # ---- cumulative semaphore targets ----
        # dve_stats: mid DVE slabs 16 each, then 8 (g12), 5 (g13), 5 (g14), 5 (g15)
        DS_MID = {0: 16, 2: 32, 4: 48, 5: 64}
        DS_G12, DS_G13, DS_G14, DS_G15 = 69, 74, 79, 84
        # mv_ready: 2 per mid DVE slab, 1 per endgame group
        MV_MID = {0: 2, 2: 4, 4: 6, 5: 8}
        MV_G12, MV_G13, MV_G14, MV_G15 = 9, 10, 11, 12
        # act_stats: mid ACT slabs 4 each (2 rows x Copy/Square);
        # endgame B passes 8 each (Copy,Sq,m2a,m2b,t,neg,M2a,M2b)
        AS_U1, AS_U3 = 4, 8
        AS_G12, AS_G13, AS_G14, AS_G15 = 15, 22, 29, 36
        # act_done (scalar program order):
        # U0 epi 4; U1 epi 8 -> 12; U2 epi 16; U3 epi 24; U4 28; U5 32;
        # endgame epis: 2 strided ops -> 34
        AD_TOTAL = 34

        @block.sync
        def _(sync):
            def mid_src(u):
                return x[u * 2 * P : (u + 1) * 2 * P, :].rearrange(
                    "(p m) f -> p (m f)", m=2
                )

            def eg_src(g, c0, w):
                return bass.AP(
                    tensor=x_ext,
                    offset=(EG_BASE + (g - 12) * P) * N + c0,
                    ap=[[N, P], [1, w]],
                )

            for u in range(6):
                sync.dma_start(out=xt[:, u, 0 : 2 * N], in_=mid_src(u)).then_inc(
                    dmid[u], 16
                )
            # endgame units reuse ring slots 0..3 of U0..U3
            # g12 -> slot 0 (U0, DVE-consumed)
            sync.wait_ge(dve_stats, DS_MID[0])
            sync.wait_ge(dmid[0], 16)
            sync.dma_start(
                out=xt[:, 0, A_COLS:N], in_=eg_src(12, A_COLS, B_COLS)
            ).then_inc(d12B, 16)
            sync.dma_start(out=xt[:, 0, 0:A_COLS], in_=eg_src(12, 0, A_COLS)).then_inc(
                d12A, 16
            )
            # g13 -> slot 1 (U1, ACT-consumed): B then A
            sync.wait_ge(act_stats, AS_U1)
            sync.wait_ge(dmid[1], 16)
            sync.dma_start(
                out=xt[:, 1, A_COLS:N], in_=eg_src(13, A_COLS, B_COLS)
            ).then_inc(d13B, 16)
            sync.dma_start(out=xt[:, 1, 0:A_COLS], in_=eg_src(13, 0, A_COLS)).then_inc(
                d13A, 16
            )
            # g14 -> slot 2 (U2, DVE)
            sync.wait_ge(dve_stats, DS_MID[2])
            sync.wait_ge(dmid[2], 16)
            sync.dma_start(
                out=xt[:, 2, A_COLS:N], in_=eg_src(14, A_COLS, B_COLS)
            ).then_inc(d14B, 16)
            sync.dma_start(out=xt[:, 2, 0:A_COLS], in_=eg_src(14, 0, A_COLS)).then_inc(
                d14A, 16
            )
            # g15 -> slot 3 (U3, ACT): B, A1, A2
            sync.wait_ge(act_stats, AS_U3)
            sync.wait_ge(dmid[3], 16)
            sync.dma_start(
                out=xt[:, 3, A_COLS:N], in_=eg_src(15, A_COLS, B_COLS)
            ).then_inc(d15B, 16)
            sync.dma_start(
                out=xt[:, 3, 0:1024], in_=eg_src(15, 0, 1024)
            ).then_inc(d15A1, 16)
            sync.dma_start(
                out=xt[:, 3, 1024:2048], in_=eg_src(15, 1024, 1024)
            ).then_inc(d15A2, 16)
            sync.dma_start(
                out=xt[:, 3, 2048:A_COLS], in_=eg_src(15, 2048, 512)
            ).then_inc(d15A3, 16)

            # final output DMAs for g12..g15 via T
            sync.wait_ge(trans_done, 5)
            dst = bass.AP(tensor=out_ext, offset=3 * 2 * C, ap=[[C, 2], [1, C]])
            sync.dma_start(out=dst, in_=T[0:8, 0:P]).then_inc(dma_fin, 16)
            sync.wait_ge(dma_out, 6 * 16)
            sync.wait_ge(dma_fin, 16)

        @block.vector
        def _(vector):
            # init: zero statsE (incl. synthetic zero fields), synth counts=768,
            # zero F. trans_done=1 signals all init memsets retired.
            vector.memset(statsE[:, :, :, :], 0.0)
            for e in range(4):
                vector.memset(statsE[:, e, 5, 0:1], float(B_COLS // 2))
                vector.memset(statsE[:, e, 5, 3:4], float(B_COLS // 2))
            vector.memset(F[:, :], 0.0).then_inc(trans_done, 1)

            ds = 0
            # mid DVE slabs
            for si, u in enumerate([0, 2, 4, 5]):
                vector.wait_ge(dmid[u], 16)
                for r in range(2):
                    for g in range(G):
                        vector.bn_stats(
                            out=stats[:, si, r, g, :],
                            in_=xt[:, u, (r * G + g) * 512 : (r * G + g + 1) * 512],
                        ).then_inc(dve_stats, 1)
                ds += 16
                vector.wait_ge(dve_stats, ds)
                for r in range(2):
                    vector.bn_aggr(
                        out=mv[:, si, r, :], in_=stats[:, si, r, :, :]
                    ).then_inc(mv_ready, 1)

            # g12 / g13 / g14 A-parts
            for e, (dA, slot, as_tgt) in enumerate(
                [(d12A, 0, AS_G12), (d13A, 1, AS_G13), (d14A, 2, AS_G14)]
            ):
                vector.wait_ge(dA, 16)
                for g in range(5):
                    vector.bn_stats(
                        out=statsE[:, e, g, :],
                        in_=xt[:, slot, g * 512 : (g + 1) * 512],
                    ).then_inc(dve_stats, 1)
                ds += 5
                vector.wait_ge(dve_stats, ds)
                vector.wait_ge(act_stats, as_tgt)  # synthetic entry written
                vector.bn_aggr(out=mvE[:, e, :], in_=statsE[:, e, 0:6, :]).then_inc(
                    mv_ready, 1
                )

            # g15: A chunks [1024, 1024, 512]
            for dchunk, ks in [(d15A1, (0, 2)), (d15A2, (2, 4)), (d15A3, (4, 5))]:
                vector.wait_ge(dchunk, 16)
                for g in range(*ks):
                    vector.bn_stats(
                        out=statsE[:, 3, g, :], in_=xt[:, 3, g * 512 : (g + 1) * 512]
                    ).then_inc(dve_stats, 1)
            ds += 5
            vector.wait_ge(dve_stats, ds)
            vector.wait_ge(act_stats, AS_G15)
            vector.bn_aggr(out=mvE[:, 3, :], in_=statsE[:, 3, 0:6, :]).then_inc(
                mv_ready, 1
            )

            # transposes F -> T once every endgame epilogue has written F
            vector.wait_ge(act_done, AD_TOTAL)
            for blk in range(P // 32):
                vector.transpose(
                    out=T[0:32, blk * 32 : blk * 32 + 32],
                    in_=F[blk * 32 : blk * 32 + 32, 0:32],
                ).then_inc(trans_done, 1)

        @block.scalar
        def _(scalar):
            ad = [0]  # running act_done
            ast = [0]  # running act_stats

            def mid_dve_epi(u, si, mv_tgt):
                scalar.wait_ge(mv_ready, mv_tgt)
                for r in range(2):
                    scalar.copy(out=res[:, u, 0, r : r + 1], in_=mv[:, si, r, 0:1]).then_inc(
                        act_done, 1
                    )
                    scalar.activation(
                        out=res[:, u, 1, r : r + 1],
                        in_=mv[:, si, r, 1:2],
                        func=AF.Sqrt,
                        scale=float(N) / (N - 1),
                    ).then_inc(act_done, 1)
                ad[0] += 4

            def mid_out_dma(u):
                b, c0 = divmod(u * 2 * P, C)
                scalar.wait_ge(act_done, ad[0])
                dst = bass.AP(
                    tensor=out_ext,
                    offset=b * 2 * C + c0,
                    ap=[[2, P], [C, 2], [1, 2]],
                )
                scalar.dma_start(out=dst, in_=res[:, u, :, 0:2]).then_inc(dma_out, 16)

            def mid_act(u, k):
                scalar.wait_ge(dmid[u], 16)
                for r in range(2):
                    row = xt[:, u, r * N : (r + 1) * N]
                    scalar.activation(
                        out=row, in_=row, func=AF.Copy, accum_out=accM[:, k, r, 0:1]
                    ).then_inc(act_stats, 1)
                    ast[0] += 1
                    scalar.wait_ge(act_stats, ast[0])
                    scalar.activation(
                        out=row, in_=row, func=AF.Square, accum_out=accM[:, k, r, 1:2]
                    ).then_inc(act_stats, 1)
                    ast[0] += 1
                # epilogue from raw sums
                for r in range(2):
                    scalar.activation(
                        out=res[:, u, 0, r : r + 1],
                        in_=accM[:, k, r, 0:1],
                        func=AF.Copy,
                        scale=1.0 / N,
                    ).then_inc(act_done, 1)
                    scalar.activation(
                        out=accM[:, k, r, 2:3],
                        in_=accM[:, k, r, 0:1],
                        func=AF.Square,
                        scale=NA,
                    ).then_inc(act_done, 1)
                    ad[0] += 2
                    scalar.wait_ge(act_done, ad[0])
                    scalar.activation(
                        out=accM[:, k, r, 2:3],
                        in_=accM[:, k, r, 2:3],
                        func=AF.Copy,
                        scale=-1.0,
                    ).then_inc(act_done, 1)
                    ad[0] += 1
                    scalar.wait_ge(act_done, ad[0])
                    scalar.activation(
                        out=res[:, u, 1, r : r + 1],
                        in_=accM[:, k, r, 1:2],
                        func=AF.Sqrt,
                        scale=1.0 / (N - 1),
                        bias=accM[:, k, r, 2:3],
                    ).then_inc(act_done, 1)
                    ad[0] += 1

            def eg_b_pass(e, dB, slot):
                """B-part accumulate + synthetic bn_stats entry for group 12+e."""
                k = e  # accE index
                scalar.wait_ge(dB, 16)
                seg = xt[:, slot, A_COLS:N]
                scalar.activation(
                    out=scr[:, :], in_=seg, func=AF.Copy, accum_out=accE[:, k, 0:1]
                ).then_inc(act_stats, 1)
                # Square with scale 1/sqrt(2) accumulates sumsqB/2 directly
                scalar.activation(
                    out=scr[:, :],
                    in_=seg,
                    func=AF.Square,
                    scale=1.0 / np.sqrt(2.0),
                    accum_out=accE[:, k, 1:2],
                ).then_inc(act_stats, 1)
                ast[0] += 2
                # synthetic entry: two triples [768, meanB, M2B/2]
                if e == 0:
                    scalar.wait_ge(trans_done, 1)  # statsE memsets retired
                scalar.wait_ge(act_stats, ast[0])  # sums retired
                scalar.activation(
                    out=statsE[:, e, 5, 1:2],
                    in_=accE[:, k, 0:1],
                    func=AF.Copy,
                    scale=1.0 / B_COLS,
                ).then_inc(act_stats, 1)
                scalar.activation(
                    out=statsE[:, e, 5, 4:5],
                    in_=accE[:, k, 0:1],
                    func=AF.Copy,
                    scale=1.0 / B_COLS,
                ).then_inc(act_stats, 1)
                # tmp = sumB^2 / (2*B_COLS)
                scalar.activation(
                    out=accE[:, k, 2:3],
                    in_=accE[:, k, 0:1],
                    func=AF.Square,
                    scale=1.0 / np.sqrt(2.0 * B_COLS),
                ).then_inc(act_stats, 1)
                ast[0] += 3
                scalar.wait_ge(act_stats, ast[0])
                # M2B/2 = sumsqB/2 - sumB^2/(2*B_COLS)
                scalar.activation(
                    out=statsE[:, e, 5, 2:3],
                    in_=accE[:, k, 2:3],
                    func=AF.Identity,
                    scale=-1.0,
                    bias=accE[:, k, 1:2],
                ).then_inc(act_stats, 1)
                scalar.activation(
                    out=statsE[:, e, 5, 5:6],
                    in_=accE[:, k, 2:3],
                    func=AF.Identity,
                    scale=-1.0,
                    bias=accE[:, k, 1:2],
                ).then_inc(act_stats, 1)
                ast[0] += 2

            def eg_epis():
                # all four endgame groups in two strided ops
                scalar.wait_ge(mv_ready, MV_G15)
                scalar.copy(
                    out=F[:, 0:4],
                    in_=mvE[:, 0:4, 0:1].rearrange("p a b -> p (a b)"),
                ).then_inc(act_done, 1)
                scalar.activation(
                    out=F[:, 4:8],
                    in_=mvE[:, 0:4, 1:2].rearrange("p a b -> p (a b)"),
                    func=AF.Sqrt,
                    scale=float(N) / (N - 1),
                ).then_inc(act_done, 1)
                ad[0] += 2

            mid_dve_epi(0, 0, MV_MID[0])
            mid_out_dma(0)
            mid_act(1, 0)
            mid_out_dma(1)
            mid_dve_epi(2, 1, MV_MID[2])
            mid_out_dma(2)
            mid_act(3, 1)
            mid_out_dma(3)
            mid_dve_epi(4, 2, MV_MID[4])
            mid_out_dma(4)
            # endgame: B passes take priority over epilogues
            eg_b_pass(0, d12B, 0)
            mid_dve_epi(5, 3, MV_MID[5])
            mid_out_dma(5)
            eg_b_pass(1, d13B, 1)
            eg_b_pass(2, d14B, 2)
            eg_b_pass(3, d15B, 3)
            eg_epis()
            assert ad[0] == AD_TOTAL, ad[0]
            assert ast[0] == AS_G15, ast[0]

    return nc


def kernel(f_vol: np.ndarray) -> np.ndarray:
    from concourse.bass_utils import run_bass_kernel_spmd

    if "nc" not in _CACHE:
        _CACHE["nc"] = _build()
    nc = _CACHE["nc"]

    f_vol = np.ascontiguousarray(f_vol, dtype=np.float32)
    in_maps = [
        {"f_vol": f_vol[i * B_LOCAL : (i + 1) * B_LOCAL]} for i in range(N_CORES)
    ]
    res = run_bass_kernel_spmd(nc, in_maps, core_ids=list(range(N_CORES)))
    return np.concatenate([res.results[i]["out"] for i in range(N_CORES)], axis=0)
